# revision 1
# baseline (speedup 1.0000x reference)
"""LinearCapsPro forward on 8 TRN2 NeuronCores.

Math: out[b,c] = sqrt(u^T sigma u), u = W_c x_b, sigma = (W_c W_c^T + eps I)^-1.
Host-side fold: G_c = W_c W_c^T + eps I = L_c L_c^T  =>  u^T G^-1 u = ||L_c^-1 u||^2.
With W'_c = L_c^-1 W_c the device kernel is just v = x @ W'^T, then
out[b,c] = sqrt(sum_d v[b, c*16+d]^2) - one big matmul + square + group-sum + sqrt.

Sharding: data-parallel over batch (512 rows/core), W' replicated; no collectives.
"""

import sys

import numpy as np
import ml_dtypes

try:
    import concourse  # noqa: F401
except ImportError:  # fresh grading dir: concourse lives in the RL repo
    sys.path.insert(0, "/opt/trn_rl_repo")

B, F, C, D = 4096, 2048, 100, 16
N_CORES = 8
BL = B // N_CORES  # 512 batch rows per core
CD = C * D  # 1600
EPS = 1e-4
KT = F // 128  # 16 contraction tiles
MT = BL // 128  # 4 batch tiles per core
N_TILES = [(0, 512), (512, 512), (1024, 512), (1536, 64)]  # cd-tiles

_cached_nc = None


def build_bass():
    import concourse.bacc as bacc
    import concourse.mybir as mybir
    import concourse.tile as tile

    nc = bacc.Bacc("TRN2", target_bir_lowering=False, debug=False, num_devices=N_CORES)
    xT = nc.dram_tensor("xT", [F, BL], mybir.dt.bfloat16, kind="ExternalInput")
    wT = nc.dram_tensor("wT", [F, CD], mybir.dt.bfloat16, kind="ExternalInput")
    out = nc.dram_tensor("out", [BL, C], mybir.dt.float32, kind="ExternalOutput")

    with tile.TileContext(nc) as tc:
        with (
            tc.tile_pool(name="xp", bufs=1) as xp,
            tc.tile_pool(name="wp", bufs=1) as wp,
            tc.tile_pool(name="ps", bufs=4, space="PSUM") as psp,
            tc.tile_pool(name="ep", bufs=4) as ep,
        ):
            xs, ws = [], []
            for k in range(KT):
                xk = xp.tile([128, BL], mybir.dt.bfloat16, tag=f"x{k}")
                nc.sync.dma_start(xk[:], xT[k * 128 : (k + 1) * 128, :])
                wk = wp.tile([128, CD], mybir.dt.bfloat16, tag=f"w{k}")
                nc.sync.dma_start(wk[:], wT[k * 128 : (k + 1) * 128, :])
                xs.append(xk)
                ws.append(wk)
            for m in range(MT):
                for noff, nsz in N_TILES:
                    ps = psp.tile([128, nsz], mybir.dt.float32, tag="ps")
                    for k in range(KT):
                        nc.tensor.matmul(
                            ps[:],
                            xs[k][:, m * 128 : (m + 1) * 128],  # lhsT [K=128, M=128]
                            ws[k][:, noff : noff + nsz],  # rhs  [K=128, N]
                            start=(k == 0),
                            stop=(k == KT - 1),
                        )
                    ncaps = nsz // D
                    sq = ep.tile([128, nsz], mybir.dt.float32, tag="sq")
                    nc.scalar.square(sq[:], ps[:])
                    red = ep.tile([128, ncaps], mybir.dt.float32, tag="red")
                    nc.vector.reduce_sum(
                        red[:],
                        sq[:].rearrange("p (c d) -> p c d", d=D),
                        axis=mybir.AxisListType.X,
                    )
                    res = ep.tile([128, ncaps], mybir.dt.float32, tag="res")
                    nc.scalar.sqrt(res[:], red[:])
                    nc.sync.dma_start(
                        out[m * 128 : (m + 1) * 128, noff // D : (noff + nsz) // D],
                        res[:],
                    )
    nc.compile()
    return nc


def prep_inputs(x: np.ndarray, weight: np.ndarray):
    """Host-side fold + shard. Returns in_maps for the 8 cores."""
    W64 = weight.astype(np.float64)  # [C, D, F]
    G = np.einsum("cdf,cef->cde", W64, W64)
    G[:, np.arange(D), np.arange(D)] += EPS
    L = np.linalg.cholesky(G)
    Wp = np.linalg.solve(L, W64)  # L^-1 W : [C, D, F]
    wT = np.ascontiguousarray(
        Wp.reshape(CD, F).T.astype(ml_dtypes.bfloat16)
    )  # [F, CD]
    xT = np.ascontiguousarray(x.T.astype(ml_dtypes.bfloat16))  # [F, B]
    return [
        {"xT": np.ascontiguousarray(xT[:, i * BL : (i + 1) * BL]), "wT": wT}
        for i in range(N_CORES)
    ]


def kernel(x: np.ndarray, weight: np.ndarray) -> np.ndarray:
    global _cached_nc
    x = np.asarray(x)
    weight = np.asarray(weight)
    assert x.shape == (B, F) and weight.shape == (C, D, F), (x.shape, weight.shape)
    in_maps = prep_inputs(x, weight)
    if _cached_nc is None:
        _cached_nc = build_bass()
    from concourse.bass_utils import run_bass_kernel_spmd

    res = run_bass_kernel_spmd(_cached_nc, in_maps, core_ids=list(range(N_CORES)))
    return np.concatenate(
        [res.results[i]["out"] for i in range(N_CORES)], axis=0
    ).astype(np.float32)


# revision 4
# speedup vs baseline: 1.0783x; 1.0783x over previous
"""LinearCapsPro forward on 8 TRN2 NeuronCores.

Math: out[b,c] = sqrt(u^T sigma u), u = W_c x_b, sigma = (W_c W_c^T + eps I)^-1.
Host-side fold: G_c = W_c W_c^T + eps I = L_c L_c^T  =>  u^T G^-1 u = ||L_c^-1 u||^2.
With W'_c = L_c^-1 W_c the device kernel is just v = x @ W'^T, then
out[b,c] = sqrt(sum_d v[b, c*16+d]^2) - one big matmul + square + group-sum + sqrt.

Sharding: data-parallel over batch (512 rows/core), W' replicated; no collectives.

Schedule (per core):
  - x^T [2048,512] bf16 loaded as 16 k-pieces on the ACT HW-DGE ring.
  - W'^T [2048,1600] bf16 loaded as 32 (k, col-half) pieces on the SP ring,
    first-half columns first so stripe-0 compute can start ~2us in.
  - Compute loops stripe(4 x 400 cd-cols) -> k(16) -> m(4 x 128 batch rows):
    4 PSUM banks live per stripe (double-buffered across stripes = 8 banks).
  - Epilogue per (stripe, m): ACT square psum->sbuf, DVE group-sum(16) into a
    per-m [128,100] result tile; final ACT sqrt + one output DMA per m.
"""

import sys

import numpy as np
import ml_dtypes

try:
    import concourse  # noqa: F401
except ImportError:  # fresh grading dir: concourse lives in the RL repo
    sys.path.insert(0, "/opt/trn_rl_repo")

B, F, C, D = 4096, 2048, 100, 16
N_CORES = 8
BL = B // N_CORES  # 512 batch rows per core
CD = C * D  # 1600
EPS = 1e-4
KT = F // 128  # 16 contraction tiles
MT = BL // 128  # 4 batch tiles per core
NS = 400  # cd-stripe width (uniform; 4 stripes; 25 capsules each)
ST = CD // NS  # 4 stripes

_cached_nc = None


def build_bass():
    import concourse.bacc as bacc
    import concourse.mybir as mybir
    import concourse.tile as tile

    bf16 = mybir.dt.bfloat16
    f32 = mybir.dt.float32
    nc = bacc.Bacc("TRN2", target_bir_lowering=False, debug=False, num_devices=N_CORES)
    xT = nc.dram_tensor("xT", [F, BL], bf16, kind="ExternalInput")
    wT = nc.dram_tensor("wT", [F, CD], bf16, kind="ExternalInput")
    out = nc.dram_tensor("out", [BL, C], f32, kind="ExternalOutput")

    with tile.TileContext(nc) as tc:
        with (
            tc.tile_pool(name="xp", bufs=1) as xp,
            tc.tile_pool(name="wp", bufs=1) as wp,
            tc.tile_pool(name="ps", bufs=2, space="PSUM") as psp,
            tc.tile_pool(name="ep", bufs=4) as ep,
            tc.tile_pool(name="rp", bufs=1) as rp,
        ):
            # x: one SBUF tile [128, 16, 512], filled by 16 k-piece DMAs on
            # the ACT ring (separate from w's SP ring so they don't serialize)
            xsb = xp.tile([128, KT, BL], bf16)
            for k in range(KT):
                nc.scalar.dma_start(xsb[:, k, :], xT[k * 128 : (k + 1) * 128, :])
            # w: one SBUF tile [128, 16, 1600]; 32 half-row pieces, low cols
            # first so stripe 0 can start while stripes 2-3 data still loads
            wsb = wp.tile([128, KT, CD], bf16)
            for h in range(2):
                for k in range(KT):
                    nc.sync.dma_start(
                        wsb[:, k, h * 800 : (h + 1) * 800],
                        wT[k * 128 : (k + 1) * 128, h * 800 : (h + 1) * 800],
                    )
            res = [
                rp.tile([128, C], f32, tag=f"res{m}", name=f"res{m}")
                for m in range(MT)
            ]
            for s in range(ST):
                noff = s * NS
                pss = [
                    psp.tile([128, NS], f32, tag=f"ps{m}", name=f"ps_s{s}_m{m}")
                    for m in range(MT)
                ]
                for k in range(KT):
                    for m in range(MT):
                        nc.tensor.matmul(
                            pss[m][:],
                            xsb[:, k, m * 128 : (m + 1) * 128],  # lhsT [K, M]
                            wsb[:, k, noff : noff + NS],  # rhs [K, N]
                            start=(k == 0),
                            stop=(k == KT - 1),
                        )
                ncaps = NS // D  # 25
                for m in range(MT):
                    sq = ep.tile([128, NS], f32, tag="sq")
                    nc.scalar.square(sq[:], pss[m][:])
                    nc.vector.reduce_sum(
                        res[m][:, s * ncaps : (s + 1) * ncaps],
                        sq[:].rearrange("p (c d) -> p c d", d=D),
                        axis=mybir.AxisListType.X,
                    )
            for m in range(MT):
                nc.scalar.sqrt(res[m][:], res[m][:])
                nc.gpsimd.dma_start(out[m * 128 : (m + 1) * 128, :], res[m][:])
    nc.compile()
    return nc


def prep_inputs(x: np.ndarray, weight: np.ndarray):
    """Host-side fold + shard. Returns in_maps for the 8 cores."""
    W64 = weight.astype(np.float64)  # [C, D, F]
    G = np.einsum("cdf,cef->cde", W64, W64)
    G[:, np.arange(D), np.arange(D)] += EPS
    L = np.linalg.cholesky(G)
    Wp = np.linalg.solve(L, W64)  # L^-1 W : [C, D, F]
    wT = np.ascontiguousarray(
        Wp.reshape(CD, F).T.astype(ml_dtypes.bfloat16)
    )  # [F, CD]
    xT = np.ascontiguousarray(x.T.astype(ml_dtypes.bfloat16))  # [F, B]
    return [
        {"xT": np.ascontiguousarray(xT[:, i * BL : (i + 1) * BL]), "wT": wT}
        for i in range(N_CORES)
    ]


def kernel(x: np.ndarray, weight: np.ndarray) -> np.ndarray:
    global _cached_nc
    x = np.asarray(x)
    weight = np.asarray(weight)
    assert x.shape == (B, F) and weight.shape == (C, D, F), (x.shape, weight.shape)
    in_maps = prep_inputs(x, weight)
    if _cached_nc is None:
        _cached_nc = build_bass()
    from concourse.bass_utils import run_bass_kernel_spmd

    res = run_bass_kernel_spmd(_cached_nc, in_maps, core_ids=list(range(N_CORES)))
    return np.concatenate(
        [res.results[i]["out"] for i in range(N_CORES)], axis=0
    ).astype(np.float32)


# revision 7
# speedup vs baseline: 1.2842x; 1.1910x over previous
"""LinearCapsPro forward on 8 TRN2 NeuronCores.

Math: out[b,c] = sqrt(u^T sigma u), u = W_c x_b, sigma = (W_c W_c^T + eps I)^-1.
Host-side fold: G_c = W_c W_c^T + eps I = L_c L_c^T  =>  u^T G^-1 u = ||L_c^-1 u||^2.
With W'_c = L_c^-1 W_c the device kernel is just v = x @ W'^T, then
out[b,c] = sqrt(sum_d v[b, c*16+d]^2) - one big matmul + square + group-sum + sqrt.

Sharding: data-parallel over batch (512 rows/core), W' replicated; no collectives.

Schedule (per core):
  - x^T [2048,512] bf16 loaded as 16 k-pieces on the ACT HW-DGE ring.
  - W'^T [2048,1600] bf16 loaded as 32 (k, col-half) pieces on the SP ring,
    first-half columns first so stripe-0 compute can start ~2us in.
  - Compute loops stripe(4 x 400 cd-cols) -> k(16) -> m(4 x 128 batch rows):
    4 PSUM banks live per stripe (double-buffered across stripes = 8 banks).
  - Epilogue per (stripe, m): ACT square psum->sbuf, DVE group-sum(16) into a
    per-m [128,100] result tile; final ACT sqrt + one output DMA per m.
"""

import sys

import numpy as np
import ml_dtypes

try:
    import concourse  # noqa: F401
except ImportError:  # fresh grading dir: concourse lives in the RL repo
    sys.path.insert(0, "/opt/trn_rl_repo")

B, F, C, D = 4096, 2048, 100, 16
N_CORES = 8
BL = B // N_CORES  # 512 batch rows per core
CD = C * D  # 1600
EPS = 1e-4
KT = F // 128  # 16 contraction tiles
MT = BL // 128  # 4 batch tiles per core
NS = 400  # cd-stripe width (uniform; 4 stripes; 25 capsules each)
ST = CD // NS  # 4 stripes

_cached_nc = None


def build_bass():
    import concourse.bacc as bacc
    import concourse.mybir as mybir
    import concourse.tile as tile

    fp16 = mybir.dt.float16
    f32 = mybir.dt.float32
    nc = bacc.Bacc("TRN2", target_bir_lowering=False, debug=False, num_devices=N_CORES)
    xT = nc.dram_tensor("xT", [F, BL], fp16, kind="ExternalInput")
    wT = nc.dram_tensor("wT", [F, CD], fp16, kind="ExternalInput")
    out = nc.dram_tensor("out", [BL, C], f32, kind="ExternalOutput")

    with tile.TileContext(nc) as tc:
        with (
            tc.tile_pool(name="xp", bufs=1) as xp,
            tc.tile_pool(name="wp", bufs=1) as wp,
            tc.tile_pool(name="ps", bufs=2, space="PSUM") as psp,
            tc.tile_pool(name="ep", bufs=4) as ep,
            tc.tile_pool(name="rp", bufs=1) as rp,
        ):
            # x: one 2MB DMA [128, 16, 512] on the ACT ring (separate from
            # w's SP ring so the two loads run in parallel)
            xsb = xp.tile([128, KT, BL], fp16)
            nc.scalar.dma_start(xsb[:], xT.rearrange("(k p) m -> p k m", p=128))
            # w: stripe-major SBUF layout [128, stripe, k, 400]; one 1.6MB DMA
            # per stripe so stripe-0 compute starts ~6us in while stripes 1-3
            # keep loading behind it on the same FIFO ring
            wsb = wp.tile([128, ST, KT, NS], fp16)
            for s in range(ST):
                nc.sync.dma_start(
                    wsb[:, s, :, :],
                    wT[:, s * NS : (s + 1) * NS].rearrange("(k p) n -> p k n", p=128),
                )
            res = [
                rp.tile([128, C], f32, tag=f"res{m}", name=f"res{m}")
                for m in range(MT)
            ]
            for s in range(ST):
                pss = [
                    psp.tile([128, NS], f32, tag=f"ps{m}", name=f"ps_s{s}_m{m}")
                    for m in range(MT)
                ]
                for k in range(KT):
                    for m in range(MT):
                        nc.tensor.matmul(
                            pss[m][:],
                            xsb[:, k, m * 128 : (m + 1) * 128],  # lhsT [K, M]
                            wsb[:, s, k, :],  # rhs [K, N]
                            start=(k == 0),
                            stop=(k == KT - 1),
                        )
                ncaps = NS // D  # 25
                for m in range(MT):
                    sq = ep.tile([128, NS], f32, tag="sq")
                    nc.scalar.square(sq[:], pss[m][:])
                    nc.vector.reduce_sum(
                        res[m][:, s * ncaps : (s + 1) * ncaps],
                        sq[:].rearrange("p (c d) -> p c d", d=D),
                        axis=mybir.AxisListType.X,
                    )
            for m in range(MT):
                nc.scalar.sqrt(res[m][:], res[m][:])
                nc.scalar.dma_start(out[m * 128 : (m + 1) * 128, :], res[m][:])
    nc.compile()
    return nc


def prep_inputs(x: np.ndarray, weight: np.ndarray):
    """Host-side fold + shard. Returns in_maps for the 8 cores."""
    W64 = weight.astype(np.float64)  # [C, D, F]
    G = np.einsum("cdf,cef->cde", W64, W64)
    G[:, np.arange(D), np.arange(D)] += EPS
    L = np.linalg.cholesky(G)
    Wp = np.linalg.solve(L, W64)  # L^-1 W : [C, D, F]
    wT = np.ascontiguousarray(Wp.reshape(CD, F).T.astype(np.float16))  # [F, CD]
    xT = np.ascontiguousarray(x.T.astype(np.float16))  # [F, B]
    return [
        {"xT": np.ascontiguousarray(xT[:, i * BL : (i + 1) * BL]), "wT": wT}
        for i in range(N_CORES)
    ]


def kernel(x: np.ndarray, weight: np.ndarray) -> np.ndarray:
    global _cached_nc
    x = np.asarray(x)
    weight = np.asarray(weight)
    assert x.shape == (B, F) and weight.shape == (C, D, F), (x.shape, weight.shape)
    in_maps = prep_inputs(x, weight)
    if _cached_nc is None:
        _cached_nc = build_bass()
    from concourse.bass_utils import run_bass_kernel_spmd

    res = run_bass_kernel_spmd(_cached_nc, in_maps, core_ids=list(range(N_CORES)))
    return np.concatenate(
        [res.results[i]["out"] for i in range(N_CORES)], axis=0
    ).astype(np.float32)
